# revision 2
# baseline (speedup 1.0000x reference)
"""DepthLSTM Trainium2 kernel.

Problem: x (32, 256, 4096) f32; per-channel scalar LSTM (input_size=1,
hidden_size=1, no bias), gate order [i, f, g, o], weights W_ih/W_hh (256, 4).
Output h for every timestep: (32, 256, 4096).

Sharding: 8 cores, core i owns channels [32*i, 32*(i+1)), all 32 batches.
Per-core layout ("gates on partitions"):
  - partitions 0..127 = (gate_group, channel): [i: 0-31, f: 32-63, o: 64-95,
    g: 96-127], channel c within each 32-row group; free dim = batch (32).
  - gate pre-activations z = W_ih*x + W_hh*h formed in PSUM by TensorE:
    a bulk "A" matmul per 512-col PSUM bank (16 timesteps) with a sparse
    lhsT [32,128] holding W_ih on block-diagonals, then one accumulate
    matmul per step adding W_hh * h_{t-1}.
  - ScalarE: sigmoid on partitions 0..95 -> SBUF; tanh(g) and tanh(c) into
    a PSUM scratch bank (walrus requires equal SBUF base partitions for
    two-SBUF-input TensorTensor ops; SBUF x PSUM pairs may differ, so the
    tanh outputs and the cell state live in PSUM scratch).
  - VectorE: cell update (i*g, f*c, add -> c in PSUM) and h = o*tanh(c)
    into an SBUF history tile that doubles as the next step's matmul rhs
    and the DMA-out staging buffer.
"""

import sys

sys.path.insert(0, "/opt/trn_rl_repo")

from contextlib import ExitStack

import numpy as np

import concourse.bacc as bacc
import concourse.tile as tile
from concourse import mybir
from concourse.bass_utils import run_bass_kernel_spmd

F32 = mybir.dt.float32
AF = mybir.ActivationFunctionType
ALU = mybir.AluOpType

B, C, T = 32, 256, 4096
N_CORES = 8
CH = C // N_CORES  # 32 channels per core
TB = 64  # timesteps per block
STEPS_PER_BANK = 16  # 512 f32 per PSUM bank / 32 batch cols

_CACHE = {}


def build_nc(t_total=T, tb=TB):
    nc = bacc.Bacc("TRN2", target_bir_lowering=False, debug=False)

    x_d = nc.dram_tensor("xt", [CH, B, t_total], F32, kind="ExternalInput").ap()
    wih_d = nc.dram_tensor("wih", [CH, 128], F32, kind="ExternalInput").ap()
    whh_d = nc.dram_tensor("whh", [CH, 128], F32, kind="ExternalInput").ap()
    out_d = nc.dram_tensor("out", [CH, B, t_total], F32, kind="ExternalOutput").ap()

    n_blocks = t_total // tb
    banks_per_block = tb // STEPS_PER_BANK

    with tile.TileContext(nc) as tc, ExitStack() as ctx:
        consts = ctx.enter_context(tc.tile_pool(name="consts", bufs=1))
        pstate = ctx.enter_context(tc.tile_pool(name="pstate", bufs=1, space="PSUM"))
        xpool = ctx.enter_context(tc.tile_pool(name="xpool", bufs=2))
        hpool = ctx.enter_context(tc.tile_pool(name="hpool", bufs=3))
        spool = ctx.enter_context(tc.tile_pool(name="spool", bufs=4))
        tpool = ctx.enter_context(tc.tile_pool(name="tpool", bufs=4))
        zpool = ctx.enter_context(
            tc.tile_pool(name="zpool", bufs=banks_per_block + 2, space="PSUM")
        )

        wih_t = consts.tile([CH, 128], F32)
        nc.sync.dma_start(wih_t[:], wih_d)
        whh_t = consts.tile([CH, 128], F32)
        nc.sync.dma_start(whh_t[:], whh_d)

        # PSUM scratch (single bank): cols 0-31 g=tanh(zg), 32-63 c state,
        # 64-95 tanh(c). All at partition base 0 so SBUF x PSUM TensorTensor
        # pairs are legal at any SBUF base.
        scratch = pstate.tile([CH, 96], F32)
        g_ps = scratch[:, 0:32]
        c_ps = scratch[:, 32:64]
        tc_ps = scratch[:, 64:96]

        h_prev_ap = None  # strided [CH, B] AP of h_{t-1} in some h_hist tile

        for blk in range(n_blocks):
            t0 = blk * tb
            x_t = xpool.tile([CH, B * tb], F32, tag="xblk")
            nc.sync.dma_start(
                x_t[:].rearrange("p (j t) -> p j t", j=B),
                x_d[:, :, t0 : t0 + tb],
            )
            x_v = x_t[:].rearrange("p (j t) -> p j t", j=B)

            h_hist = hpool.tile([CH, B * tb], F32, tag="hblk")
            h_v = h_hist[:].rearrange("p (j t) -> p j t", j=B)

            z_banks = []
            for bank in range(banks_per_block):
                z_ps = zpool.tile([128, 512], F32, tag="zbank")
                # A = W_ih * x for 16 steps: rhs free dims (t, j)
                rhs = x_v[:, :, bank * 16 : (bank + 1) * 16].transpose([0, 2, 1])
                nc.tensor.matmul(
                    z_ps[:], wih_t[:], rhs, start=True, stop=False,
                    skip_group_check=True,
                )
                z_banks.append(z_ps)

            for tl in range(tb):
                t = t0 + tl
                bank, t16 = tl // STEPS_PER_BANK, tl % STEPS_PER_BANK
                z_ps = z_banks[bank]
                zc = z_ps[:, t16 * B : (t16 + 1) * B]

                if t > 0:
                    nc.tensor.matmul(
                        zc, whh_t[:], h_prev_ap,
                        start=False, stop=(t16 == STEPS_PER_BANK - 1),
                        skip_group_check=True,
                    )

                s_t = spool.tile([96, B], F32, tag="s")
                nc.scalar.activation(s_t[:], zc[0:96, :], AF.Sigmoid)
                nc.scalar.activation(g_ps, zc[96:128, :], AF.Tanh)

                if t == 0:
                    # c = i * g
                    nc.vector.tensor_tensor(c_ps, s_t[0:32, :], g_ps, ALU.mult)
                else:
                    t1 = tpool.tile([CH, B], F32, tag="t1")
                    nc.vector.tensor_tensor(t1[:], s_t[0:32, :], g_ps, ALU.mult)
                    t2 = tpool.tile([CH, B], F32, tag="t2")
                    nc.vector.tensor_tensor(t2[:], s_t[32:64, :], c_ps, ALU.mult)
                    nc.vector.tensor_tensor(c_ps, t1[:], t2[:], ALU.add)

                nc.scalar.activation(tc_ps, c_ps, AF.Tanh)

                h_slice = h_v[:, :, tl : tl + 1]
                nc.vector.tensor_tensor(
                    h_slice, s_t[64:96, :],
                    tc_ps.rearrange("p (j one) -> p j one", one=1),
                    ALU.mult,
                )
                h_prev_ap = h_slice.rearrange("p j one -> p (j one)")

            nc.sync.dma_start(out_d[:, :, t0 : t0 + tb], h_v)

    nc.compile()
    return nc


def _build_lhst(w4):
    """w4: [CH, 4] gate order [i, f, g, o] -> sparse lhsT [CH, 128] with
    partition groups [i, f, o, g]."""
    lhst = np.zeros((CH, 128), np.float32)
    idx = np.arange(CH)
    lhst[idx, 0 * 32 + idx] = w4[:, 0]  # i
    lhst[idx, 1 * 32 + idx] = w4[:, 1]  # f
    lhst[idx, 2 * 32 + idx] = w4[:, 3]  # o
    lhst[idx, 3 * 32 + idx] = w4[:, 2]  # g
    return lhst


def kernel(x, W_ih, W_hh):
    x = np.asarray(x, np.float32)
    W_ih = np.asarray(W_ih, np.float32)
    W_hh = np.asarray(W_hh, np.float32)

    key = ("nc", T, TB)
    if key not in _CACHE:
        _CACHE[key] = build_nc(T, TB)
    nc = _CACHE[key]

    in_maps = []
    for core in range(N_CORES):
        c0 = core * CH
        xt = np.ascontiguousarray(x[:, c0 : c0 + CH, :].transpose(1, 0, 2))
        in_maps.append(
            {
                "xt": xt,
                "wih": _build_lhst(W_ih[c0 : c0 + CH]),
                "whh": _build_lhst(W_hh[c0 : c0 + CH]),
            }
        )

    res = run_bass_kernel_spmd(nc, in_maps, list(range(N_CORES)))

    out = np.empty((B, C, T), np.float32)
    for core in range(N_CORES):
        c0 = core * CH
        out[:, c0 : c0 + CH, :] = res.results[core]["out"].transpose(1, 0, 2)
    return out


# revision 10
# speedup vs baseline: 2054.5047x; 2054.5047x over previous
"""DepthLSTM Trainium2 kernel (scheme F: gates along the free dimension).

Problem: x (32, 256, 4096) f32; per-channel scalar LSTM (input_size=1,
hidden_size=1, no bias), gate order [i, f, g, o], weights W_ih/W_hh (256, 4).
Output h for every timestep: (32, 256, 4096).

Sharding: 8 cores as (channel-block, batch-block) = (2 x 4): core idx
(cb, bb) owns channels [128*cb, 128*cb+128) and batches [8*bb, 8*bb+8).

Per-core layout: partitions = 128 channels; free dim packs (gate k, batch j)
as col k*8+j, gate order [i, f, o, g]. No TensorE/PSUM at all -- the gate
pre-activation z_t = A_t + W_hh * h_{t-1} is two VectorE ops (a broadcast
tensor_tensor against a host-replicated [128, 32] weight tile, then an add
against the bulk-precomputed A = x * W_ih).

Per step (all tiles SBUF, all ops full 128 partitions):
  sig:  s = sigmoid(z)            one ScalarE op over all 4 gate groups;
                                  the g cols hold sigma(2*zg) because the
                                  host doubles the g-gate weights
                                  (tanh(x) = 2*sigma(2x) - 1).
  cell: t1 = (sg - 0.5) * i       scalar_tensor_tensor; equals i*g/2
        t2 = f * c'               with rescaled state c' = c/2
        c' = t1 + t2
  out:  tc = tanh(2*c') = tanh(c) ScalarE with scale=2
        h  = o * tc               written into the h history tile (also the
                                  DMA staging buffer and the z rhs)
  z':   zm = h_bcast * Whh_rep    stride-0 broadcast of h over the 4 gates
        z  = zm + A_{t+1}

A = x * W_ih is precomputed per T-block, split across VectorE (i, f gates)
and ScalarE (o, g gates) in quarter-block chunks so the serial per-step
dependency chain is never blocked behind a long bulk op.
"""

import sys

sys.path.insert(0, "/opt/trn_rl_repo")

from contextlib import ExitStack

import numpy as np

import concourse.bacc as bacc
import concourse.tile as tile
from concourse import mybir
from concourse.bass_utils import run_bass_kernel_spmd

F32 = mybir.dt.float32
AF = mybir.ActivationFunctionType
ALU = mybir.AluOpType

B, C, T = 32, 256, 4096
N_CORES = 8
CH = 128  # channels per core
BJ = 8  # batches per core
TB = 64  # timesteps per block

_CACHE = {}


def build_nc(t_total=T, tb=TB):
    nc = bacc.Bacc("TRN2", target_bir_lowering=False, debug=False)

    x_d = nc.dram_tensor("xt", [CH, BJ, t_total], F32, kind="ExternalInput").ap()
    wih_d = nc.dram_tensor("wih", [CH, 32], F32, kind="ExternalInput").ap()
    whh_d = nc.dram_tensor("whh", [CH, 32], F32, kind="ExternalInput").ap()
    out_d = nc.dram_tensor("out", [CH, BJ, t_total], F32, kind="ExternalOutput").ap()

    n_blocks = t_total // tb

    with tile.TileContext(nc) as tc, ExitStack() as ctx:
        consts = ctx.enter_context(tc.tile_pool(name="consts", bufs=1))
        state = ctx.enter_context(tc.tile_pool(name="state", bufs=1))
        xpool = ctx.enter_context(tc.tile_pool(name="xpool", bufs=2))
        apool = ctx.enter_context(tc.tile_pool(name="apool", bufs=2))
        hpool = ctx.enter_context(tc.tile_pool(name="hpool", bufs=3))
        spool = ctx.enter_context(tc.tile_pool(name="spool", bufs=4))
        tpool = ctx.enter_context(tc.tile_pool(name="tpool", bufs=4))

        wih_t = consts.tile([CH, 32], F32)
        nc.sync.dma_start(wih_t[:], wih_d)
        whh_t = consts.tile([CH, 32], F32)
        nc.sync.dma_start(whh_t[:], whh_d)

        c_t = state.tile([CH, BJ], F32)  # rescaled cell state c' = c/2

        tc_prev = None  # tanh(c) tile from the previous step
        ow_prev = None  # o * Whh_rep tile from the previous step

        for blk in range(n_blocks):
            t0 = blk * tb
            x_t = xpool.tile([CH, BJ * tb], F32, tag="xblk")
            nc.sync.dma_start(
                x_t[:].rearrange("p (j t) -> p j t", j=BJ),
                x_d[:, :, t0 : t0 + tb],
            )
            # x viewed as [p, t, j] to match A's (t, k, j) col order
            x_tj = x_t[:].rearrange("p (j t) -> p j t", j=BJ).transpose([0, 2, 1])

            a_t = apool.tile([CH, tb * 32], F32, tag="ablk")
            a_v = a_t[:].rearrange("p (t k j) -> p t k j", k=4, j=BJ)
            # A[:, t, k, j] = x[:, t, j] * wih[:, k*8]  (chunked, DVE + ACT)
            qt = tb // 4
            for k in range(4):
                eng = "v" if k < 2 else "a"
                for q in range(4):
                    src = x_tj[:, q * qt : (q + 1) * qt, :]
                    dst = a_v[:, q * qt : (q + 1) * qt, k, :]
                    w_col = wih_t[:, k * BJ : k * BJ + 1]
                    if eng == "v":
                        nc.vector.tensor_scalar(dst, src, w_col, None, ALU.mult)
                    else:
                        nc.scalar.activation(dst, src, AF.Copy, scale=w_col)

            h_hist = hpool.tile([CH, BJ * tb], F32, tag="hblk")
            h_v = h_hist[:].rearrange("p (j t) -> p j t", j=BJ)

            for tl in range(tb):
                t = t0 + tl

                if t == 0:
                    z_ap = a_t[:, 0:32]
                else:
                    # z = h_{t-1} (bcast over gates) * Whh_rep + A_t, computed
                    # as (o_{t-1}*Whh_rep) * tc_{t-1} + A_t: the ow product
                    # rides in the previous step's first DVE block (off the
                    # critical path), so zm here depends only on tanh(c) and
                    # issues back-to-back with the h output op.
                    zm = tpool.tile([CH, 32], F32, tag="zm")
                    tc_b = tc_prev[:].rearrange(
                        "p (one j) -> p one j", one=1
                    ).broadcast_to((CH, 4, BJ))
                    nc.vector.tensor_tensor(
                        zm[:].rearrange("p (k j) -> p k j", k=4),
                        tc_b,
                        ow_prev[:].rearrange("p (k j) -> p k j", k=4),
                        ALU.mult,
                    )
                    z_t = tpool.tile([CH, 32], F32, tag="z")
                    nc.vector.tensor_tensor(
                        z_t[:], zm[:], a_t[:, tl * 32 : tl * 32 + 32], ALU.add
                    )
                    z_ap = z_t[:]

                s_t = spool.tile([CH, 32], F32, tag="s")
                nc.scalar.activation(s_t[:], z_ap, AF.Sigmoid)
                s_i = s_t[:, 0:BJ]
                s_f = s_t[:, BJ : 2 * BJ]
                s_o = s_t[:, 2 * BJ : 3 * BJ]
                s_g = s_t[:, 3 * BJ : 4 * BJ]

                if t == 0:
                    # c' = i * g / 2 = (sg - 0.5) * i
                    nc.vector.scalar_tensor_tensor(
                        c_t[:], s_g, 0.5, s_i, ALU.subtract, ALU.mult
                    )
                else:
                    t1 = tpool.tile([CH, BJ], F32, tag="t1")
                    nc.vector.scalar_tensor_tensor(
                        t1[:], s_g, 0.5, s_i, ALU.subtract, ALU.mult
                    )
                    t2 = tpool.tile([CH, BJ], F32, tag="t2")
                    nc.vector.tensor_tensor(t2[:], s_f, c_t[:], ALU.mult)
                    nc.vector.tensor_tensor(c_t[:], t1[:], t2[:], ALU.add)

                # ow = o * Whh_rep for the NEXT step's zm; only needs sigma
                # output, so it fills this DVE block's idle tail.
                ow = tpool.tile([CH, 32], F32, tag="ow")
                nc.vector.tensor_tensor(
                    ow[:].rearrange("p (k j) -> p k j", k=4),
                    s_o.rearrange("p (one j) -> p one j", one=1).broadcast_to(
                        (CH, 4, BJ)
                    ),
                    whh_t[:].rearrange("p (k j) -> p k j", k=4),
                    ALU.mult,
                )

                tc_t = tpool.tile([CH, BJ], F32, tag="tc")
                nc.scalar.activation(tc_t[:], c_t[:], AF.Tanh, scale=2.0)

                h_slice = h_v[:, :, tl : tl + 1]
                nc.vector.tensor_tensor(
                    h_slice, s_o.rearrange("p (j one) -> p j one", one=1),
                    tc_t[:].rearrange("p (j one) -> p j one", one=1),
                    ALU.mult,
                )
                tc_prev, ow_prev = tc_t, ow

            nc.sync.dma_start(out_d[:, :, t0 : t0 + tb], h_v)

    nc.compile()
    return nc


def _build_wrep(w4):
    """w4: [CH, 4] gate order [i, f, g, o] -> [CH, 32] with col k*8+j holding
    the gate-k weight (j-independent), col gate order [i, f, o, g], g doubled
    for the tanh-to-sigmoid transform."""
    cols = np.stack(
        [w4[:, 0], w4[:, 1], w4[:, 3], 2.0 * w4[:, 2]], axis=1
    )  # [CH, 4]
    return np.ascontiguousarray(np.repeat(cols, BJ, axis=1).astype(np.float32))


def kernel(x, W_ih, W_hh):
    x = np.asarray(x, np.float32)
    W_ih = np.asarray(W_ih, np.float32)
    W_hh = np.asarray(W_hh, np.float32)

    key = ("nc", T, TB)
    if key not in _CACHE:
        _CACHE[key] = build_nc(T, TB)
    nc = _CACHE[key]

    in_maps = []
    for core in range(N_CORES):
        cb, bb = divmod(core, 4)
        c0, b0 = cb * CH, bb * BJ
        xt = np.ascontiguousarray(
            x[b0 : b0 + BJ, c0 : c0 + CH, :].transpose(1, 0, 2)
        )
        in_maps.append(
            {
                "xt": xt,
                "wih": _build_wrep(W_ih[c0 : c0 + CH]),
                "whh": _build_wrep(W_hh[c0 : c0 + CH]),
            }
        )

    res = run_bass_kernel_spmd(nc, in_maps, list(range(N_CORES)))

    out = np.empty((B, C, T), np.float32)
    for core in range(N_CORES):
        cb, bb = divmod(core, 4)
        c0, b0 = cb * CH, bb * BJ
        out[b0 : b0 + BJ, c0 : c0 + CH, :] = res.results[core]["out"].transpose(
            1, 0, 2
        )
    return out


# revision 13
# speedup vs baseline: 2134.1653x; 1.0388x over previous
"""DepthLSTM Trainium2 kernel (scheme F: gates along the free dimension).

Problem: x (32, 256, 4096) f32; per-channel scalar LSTM (input_size=1,
hidden_size=1, no bias), gate order [i, f, g, o], weights W_ih/W_hh (256, 4).
Output h for every timestep: (32, 256, 4096).

Sharding: 8 cores as (channel-block, batch-block) = (2 x 4): core idx
(cb, bb) owns channels [128*cb, 128*cb+128) and batches [8*bb, 8*bb+8).

Per-core layout: partitions = 128 channels; free dim packs (gate k, batch j)
as col k*8+j, gate order [i, f, o, g]. No TensorE/PSUM at all -- the gate
pre-activation z_t = A_t + W_hh * h_{t-1} is two VectorE ops (a broadcast
tensor_tensor against a host-replicated [128, 32] weight tile, then an add
against the bulk-precomputed A = x * W_ih).

Per step (all tiles SBUF, all ops full 128 partitions):
  sig:  s = sigmoid(z)            one ScalarE op over all 4 gate groups;
                                  the g cols hold sigma(2*zg) because the
                                  host doubles the g-gate weights
                                  (tanh(x) = 2*sigma(2x) - 1).
  cell: t1 = (sg - 0.5) * i       scalar_tensor_tensor; equals i*g/2
        t2 = f * c'               with rescaled state c' = c/2
        c' = t1 + t2
  out:  tc = tanh(2*c') = tanh(c) ScalarE with scale=2
        h  = o * tc               written into the h history tile (also the
                                  DMA staging buffer and the z rhs)
  z':   zm = h_bcast * Whh_rep    stride-0 broadcast of h over the 4 gates
        z  = zm + A_{t+1}

A = x * W_ih is precomputed per T-block, split across VectorE (i, f gates)
and ScalarE (o, g gates) in quarter-block chunks so the serial per-step
dependency chain is never blocked behind a long bulk op.
"""

import sys

sys.path.insert(0, "/opt/trn_rl_repo")

from contextlib import ExitStack

import numpy as np

import concourse.bacc as bacc
import concourse.tile as tile
from concourse import mybir
from concourse.bass_utils import run_bass_kernel_spmd

F32 = mybir.dt.float32
AF = mybir.ActivationFunctionType
ALU = mybir.AluOpType

B, C, T = 32, 256, 4096
N_CORES = 8
CH = 128  # channels per core
BJ = 8  # batches per core
TB = 64  # timesteps per block

_CACHE = {}


def build_nc(t_total=T, tb=TB):
    nc = bacc.Bacc("TRN2", target_bir_lowering=False, debug=False)

    x_d = nc.dram_tensor("xt", [CH, BJ, t_total], F32, kind="ExternalInput").ap()
    wih_d = nc.dram_tensor("wih", [CH, 32], F32, kind="ExternalInput").ap()
    whh_d = nc.dram_tensor("whh", [CH, 32], F32, kind="ExternalInput").ap()
    out_d = nc.dram_tensor("out", [CH, BJ, t_total], F32, kind="ExternalOutput").ap()

    n_blocks = t_total // tb

    with tile.TileContext(nc) as tc, ExitStack() as ctx:
        consts = ctx.enter_context(tc.tile_pool(name="consts", bufs=1))
        state = ctx.enter_context(tc.tile_pool(name="state", bufs=1))
        xpool = ctx.enter_context(tc.tile_pool(name="xpool", bufs=2))
        apool = ctx.enter_context(tc.tile_pool(name="apool", bufs=2))
        hpool = ctx.enter_context(tc.tile_pool(name="hpool", bufs=3))
        spool = ctx.enter_context(tc.tile_pool(name="spool", bufs=4))
        tpool = ctx.enter_context(tc.tile_pool(name="tpool", bufs=4))

        wih_t = consts.tile([CH, 32], F32)
        nc.sync.dma_start(wih_t[:], wih_d)
        whh_t = consts.tile([CH, 32], F32)
        nc.sync.dma_start(whh_t[:], whh_d)

        c_t = state.tile([CH, BJ], F32)  # rescaled cell state c' = c/2

        tc_prev = None  # tanh(c) tile from the previous step
        ow_prev = None  # o * Whh_rep tile from the previous step
        h_pending = None  # (h_slice, s_o, tc) for the previous step: the h
        # output op feeds only the DMA, so it is emitted after the next
        # step's z ops and runs during the sigma hop, off the critical path.
        dma_pending = None  # (out_slice, h_view) for a finished block

        for blk in range(n_blocks):
            t0 = blk * tb
            x_t = xpool.tile([CH, BJ * tb], F32, tag="xblk")
            nc.sync.dma_start(
                x_t[:].rearrange("p (j t) -> p j t", j=BJ),
                x_d[:, :, t0 : t0 + tb],
            )
            # x viewed as [p, t, j] to match A's (t, k, j) col order
            x_tj = x_t[:].rearrange("p (j t) -> p j t", j=BJ).transpose([0, 2, 1])

            a_t = apool.tile([CH, tb * 32], F32, tag="ablk")
            a_v = a_t[:].rearrange("p (t k j) -> p t k j", k=4, j=BJ)
            # A[:, t, k, j] = x[:, t, j] * wih[:, k*8]  (chunked, DVE + ACT)
            qt = tb // 4
            for k in range(4):
                eng = "v" if k < 2 else "a"
                for q in range(4):
                    src = x_tj[:, q * qt : (q + 1) * qt, :]
                    dst = a_v[:, q * qt : (q + 1) * qt, k, :]
                    w_col = wih_t[:, k * BJ : k * BJ + 1]
                    if eng == "v":
                        nc.vector.tensor_scalar(dst, src, w_col, None, ALU.mult)
                    else:
                        nc.scalar.activation(dst, src, AF.Copy, scale=w_col)

            h_hist = hpool.tile([CH, BJ * tb], F32, tag="hblk")
            h_v = h_hist[:].rearrange("p (j t) -> p j t", j=BJ)

            for tl in range(tb):
                t = t0 + tl

                if t == 0:
                    z_ap = a_t[:, 0:32]
                else:
                    # z = h_{t-1} (bcast over gates) * Whh_rep + A_t, computed
                    # as (o_{t-1}*Whh_rep) * tc_{t-1} + A_t: the ow product
                    # rides in the previous step's first DVE block (off the
                    # critical path), so zm here depends only on tanh(c) and
                    # issues back-to-back with the h output op.
                    zm = tpool.tile([CH, 32], F32, tag="zm")
                    tc_b = tc_prev[:].rearrange(
                        "p (one j) -> p one j", one=1
                    ).broadcast_to((CH, 4, BJ))
                    nc.vector.tensor_tensor(
                        zm[:].rearrange("p (k j) -> p k j", k=4),
                        tc_b,
                        ow_prev[:].rearrange("p (k j) -> p k j", k=4),
                        ALU.mult,
                    )
                    z_t = tpool.tile([CH, 32], F32, tag="z")
                    nc.vector.tensor_tensor(
                        z_t[:], zm[:], a_t[:, tl * 32 : tl * 32 + 32], ALU.add
                    )
                    z_ap = z_t[:]

                if h_pending is not None:
                    ph_slice, ps_o, ptc = h_pending
                    nc.vector.tensor_tensor(
                        ph_slice, ps_o.rearrange("p (j one) -> p j one", one=1),
                        ptc[:].rearrange("p (j one) -> p j one", one=1),
                        ALU.mult,
                    )
                    h_pending = None
                    if dma_pending is not None:
                        pout, ph_v = dma_pending
                        nc.sync.dma_start(pout, ph_v)
                        dma_pending = None

                s_t = spool.tile([CH, 32], F32, tag="s")
                nc.scalar.activation(s_t[:], z_ap, AF.Sigmoid)
                s_i = s_t[:, 0:BJ]
                s_f = s_t[:, BJ : 2 * BJ]
                s_o = s_t[:, 2 * BJ : 3 * BJ]
                s_g = s_t[:, 3 * BJ : 4 * BJ]

                if t == 0:
                    # c' = i * g / 2 = (sg - 0.5) * i
                    nc.vector.scalar_tensor_tensor(
                        c_t[:], s_g, 0.5, s_i, ALU.subtract, ALU.mult
                    )
                else:
                    t1 = tpool.tile([CH, BJ], F32, tag="t1")
                    nc.vector.scalar_tensor_tensor(
                        t1[:], s_g, 0.5, s_i, ALU.subtract, ALU.mult
                    )
                    t2 = tpool.tile([CH, BJ], F32, tag="t2")
                    nc.vector.tensor_tensor(t2[:], s_f, c_t[:], ALU.mult)
                    nc.vector.tensor_tensor(c_t[:], t1[:], t2[:], ALU.add)

                # ow = o * Whh_rep for the NEXT step's zm; only needs sigma
                # output, so it fills this DVE block's idle tail.
                ow = tpool.tile([CH, 32], F32, tag="ow")
                nc.vector.tensor_tensor(
                    ow[:].rearrange("p (k j) -> p k j", k=4),
                    s_o.rearrange("p (one j) -> p one j", one=1).broadcast_to(
                        (CH, 4, BJ)
                    ),
                    whh_t[:].rearrange("p (k j) -> p k j", k=4),
                    ALU.mult,
                )

                tc_t = tpool.tile([CH, BJ], F32, tag="tc")
                nc.scalar.activation(tc_t[:], c_t[:], AF.Tanh, scale=2.0)

                h_pending = (h_v[:, :, tl : tl + 1], s_o, tc_t)
                tc_prev, ow_prev = tc_t, ow

            dma_pending = (out_d[:, :, t0 : t0 + tb], h_v)

        # drain the last step's h and the last block's DMA
        ph_slice, ps_o, ptc = h_pending
        nc.vector.tensor_tensor(
            ph_slice, ps_o.rearrange("p (j one) -> p j one", one=1),
            ptc[:].rearrange("p (j one) -> p j one", one=1),
            ALU.mult,
        )
        pout, ph_v = dma_pending
        nc.sync.dma_start(pout, ph_v)

    nc.compile()
    return nc


def _build_wrep(w4):
    """w4: [CH, 4] gate order [i, f, g, o] -> [CH, 32] with col k*8+j holding
    the gate-k weight (j-independent), col gate order [i, f, o, g], g doubled
    for the tanh-to-sigmoid transform."""
    cols = np.stack(
        [w4[:, 0], w4[:, 1], w4[:, 3], 2.0 * w4[:, 2]], axis=1
    )  # [CH, 4]
    return np.ascontiguousarray(np.repeat(cols, BJ, axis=1).astype(np.float32))


def kernel(x, W_ih, W_hh):
    x = np.asarray(x, np.float32)
    W_ih = np.asarray(W_ih, np.float32)
    W_hh = np.asarray(W_hh, np.float32)

    key = ("nc", T, TB)
    if key not in _CACHE:
        _CACHE[key] = build_nc(T, TB)
    nc = _CACHE[key]

    in_maps = []
    for core in range(N_CORES):
        cb, bb = divmod(core, 4)
        c0, b0 = cb * CH, bb * BJ
        xt = np.ascontiguousarray(
            x[b0 : b0 + BJ, c0 : c0 + CH, :].transpose(1, 0, 2)
        )
        in_maps.append(
            {
                "xt": xt,
                "wih": _build_wrep(W_ih[c0 : c0 + CH]),
                "whh": _build_wrep(W_hh[c0 : c0 + CH]),
            }
        )

    res = run_bass_kernel_spmd(nc, in_maps, list(range(N_CORES)))

    out = np.empty((B, C, T), np.float32)
    for core in range(N_CORES):
        cb, bb = divmod(core, 4)
        c0, b0 = cb * CH, bb * BJ
        out[b0 : b0 + BJ, c0 : c0 + CH, :] = res.results[core]["out"].transpose(
            1, 0, 2
        )
    return out


# revision 14
# speedup vs baseline: 2230.5063x; 1.0451x over previous
"""DepthLSTM Trainium2 kernel (scheme F: gates along the free dimension).

Problem: x (32, 256, 4096) f32; per-channel scalar LSTM (input_size=1,
hidden_size=1, no bias), gate order [i, f, g, o], weights W_ih/W_hh (256, 4).
Output h for every timestep: (32, 256, 4096).

Sharding: 8 cores as (channel-block, batch-block) = (2 x 4): core idx
(cb, bb) owns channels [128*cb, 128*cb+128) and batches [8*bb, 8*bb+8).

Per-core layout: partitions = 128 channels; free dim packs (gate k, batch j)
as col k*8+j, gate order [i, f, o, g]. No TensorE/PSUM at all -- the gate
pre-activation z_t = A_t + W_hh * h_{t-1} is two VectorE ops (a broadcast
tensor_tensor against a host-replicated [128, 32] weight tile, then an add
against the bulk-precomputed A = x * W_ih).

Per step (all tiles SBUF, all ops full 128 partitions):
  sig:  s = sigmoid(z)            one ScalarE op over all 4 gate groups;
                                  the g cols hold sigma(2*zg) because the
                                  host doubles the g-gate weights
                                  (tanh(x) = 2*sigma(2x) - 1).
  cell: t1 = (sg - 0.5) * i       scalar_tensor_tensor; equals i*g/2
        t2 = f * c'               with rescaled state c' = c/2
        c' = t1 + t2
  out:  tc = tanh(2*c') = tanh(c) ScalarE with scale=2
        h  = o * tc               written into the h history tile (also the
                                  DMA staging buffer and the z rhs)
  z':   zm = h_bcast * Whh_rep    stride-0 broadcast of h over the 4 gates
        z  = zm + A_{t+1}

A = x * W_ih is precomputed per T-block, split across VectorE (i, f gates)
and ScalarE (o, g gates) in quarter-block chunks so the serial per-step
dependency chain is never blocked behind a long bulk op.
"""

import sys

sys.path.insert(0, "/opt/trn_rl_repo")

from contextlib import ExitStack

import numpy as np

import concourse.bacc as bacc
import concourse.tile as tile
from concourse import mybir
from concourse.bass_utils import run_bass_kernel_spmd

F32 = mybir.dt.float32
AF = mybir.ActivationFunctionType
ALU = mybir.AluOpType

B, C, T = 32, 256, 4096
N_CORES = 8
CH = 128  # channels per core
BJ = 8  # batches per core
TB = 64  # timesteps per block

_CACHE = {}


def build_nc(t_total=T, tb=TB):
    nc = bacc.Bacc("TRN2", target_bir_lowering=False, debug=False)

    x_d = nc.dram_tensor("xt", [CH, BJ, t_total], F32, kind="ExternalInput").ap()
    wih_d = nc.dram_tensor("wih", [CH, 32], F32, kind="ExternalInput").ap()
    whh_d = nc.dram_tensor("whh", [CH, 32], F32, kind="ExternalInput").ap()
    out_d = nc.dram_tensor("out", [CH, BJ, t_total], F32, kind="ExternalOutput").ap()

    n_blocks = t_total // tb

    with tile.TileContext(nc) as tc, ExitStack() as ctx:
        consts = ctx.enter_context(tc.tile_pool(name="consts", bufs=1))
        state = ctx.enter_context(tc.tile_pool(name="state", bufs=1))
        xpool = ctx.enter_context(tc.tile_pool(name="xpool", bufs=3))
        apool = ctx.enter_context(tc.tile_pool(name="apool", bufs=3))
        hpool = ctx.enter_context(tc.tile_pool(name="hpool", bufs=4))
        spool = ctx.enter_context(tc.tile_pool(name="spool", bufs=6))
        tpool = ctx.enter_context(tc.tile_pool(name="tpool", bufs=8))

        wih_t = consts.tile([CH, 32], F32)
        nc.sync.dma_start(wih_t[:], wih_d)
        whh_t = consts.tile([CH, 32], F32)
        nc.sync.dma_start(whh_t[:], whh_d)

        c_t = state.tile([CH, BJ], F32)  # rescaled cell state c' = c/2

        tc_prev = None  # tanh(c) tile from the previous step
        ow_prev = None  # o * Whh_rep tile from the previous step
        h_pending = None  # (h_slice, s_o, tc) for the previous step: the h
        # output op feeds only the DMA, so it is emitted after the next
        # step's z ops and runs during the sigma hop, off the critical path.
        dma_pending = None  # (out_slice, h_view) for a finished block

        for blk in range(n_blocks):
            t0 = blk * tb
            x_t = xpool.tile([CH, BJ * tb], F32, tag="xblk")
            nc.sync.dma_start(
                x_t[:].rearrange("p (j t) -> p j t", j=BJ),
                x_d[:, :, t0 : t0 + tb],
            )
            # x viewed as [p, t, j] to match A's (t, k, j) col order
            x_tj = x_t[:].rearrange("p (j t) -> p j t", j=BJ).transpose([0, 2, 1])

            a_t = apool.tile([CH, tb * 32], F32, tag="ablk")
            a_v = a_t[:].rearrange("p (t k j) -> p t k j", k=4, j=BJ)
            # A[:, t, k, j] = x[:, t, j] * wih[:, k*8]  (chunked, DVE + ACT)
            qt = tb // 4
            for k in range(4):
                eng = "v" if k < 2 else "a"
                for q in range(4):
                    src = x_tj[:, q * qt : (q + 1) * qt, :]
                    dst = a_v[:, q * qt : (q + 1) * qt, k, :]
                    w_col = wih_t[:, k * BJ : k * BJ + 1]
                    if eng == "v":
                        nc.vector.tensor_scalar(dst, src, w_col, None, ALU.mult)
                    else:
                        nc.scalar.activation(dst, src, AF.Copy, scale=w_col)

            h_hist = hpool.tile([CH, BJ * tb], F32, tag="hblk")
            h_v = h_hist[:].rearrange("p (j t) -> p j t", j=BJ)

            for tl in range(tb):
                t = t0 + tl

                if t == 0:
                    z_ap = a_t[:, 0:32]
                else:
                    # z = h_{t-1} (bcast over gates) * Whh_rep + A_t, computed
                    # as (o_{t-1}*Whh_rep) * tc_{t-1} + A_t: the ow product
                    # rides in the previous step's first DVE block (off the
                    # critical path), so zm here depends only on tanh(c) and
                    # issues back-to-back with the h output op.
                    zm = tpool.tile([CH, 32], F32, tag="zm")
                    tc_b = tc_prev[:].rearrange(
                        "p (one j) -> p one j", one=1
                    ).broadcast_to((CH, 4, BJ))
                    nc.vector.tensor_tensor(
                        zm[:].rearrange("p (k j) -> p k j", k=4),
                        tc_b,
                        ow_prev[:].rearrange("p (k j) -> p k j", k=4),
                        ALU.mult,
                    )
                    z_t = tpool.tile([CH, 32], F32, tag="z")
                    nc.vector.tensor_tensor(
                        z_t[:], zm[:], a_t[:, tl * 32 : tl * 32 + 32], ALU.add
                    )
                    z_ap = z_t[:]

                if h_pending is not None:
                    ph_slice, ps_o, ptc = h_pending
                    nc.vector.tensor_tensor(
                        ph_slice, ps_o.rearrange("p (j one) -> p j one", one=1),
                        ptc[:].rearrange("p (j one) -> p j one", one=1),
                        ALU.mult,
                    )
                    h_pending = None
                    if dma_pending is not None:
                        pout, ph_v = dma_pending
                        nc.sync.dma_start(pout, ph_v)
                        dma_pending = None

                s_t = spool.tile([CH, 32], F32, tag="s")
                nc.scalar.activation(s_t[:], z_ap, AF.Sigmoid)
                s_i = s_t[:, 0:BJ]
                s_f = s_t[:, BJ : 2 * BJ]
                s_o = s_t[:, 2 * BJ : 3 * BJ]
                s_g = s_t[:, 3 * BJ : 4 * BJ]

                if t == 0:
                    # c' = i * g / 2 = (sg - 0.5) * i
                    nc.vector.scalar_tensor_tensor(
                        c_t[:], s_g, 0.5, s_i, ALU.subtract, ALU.mult
                    )
                else:
                    t1 = tpool.tile([CH, BJ], F32, tag="t1")
                    nc.vector.scalar_tensor_tensor(
                        t1[:], s_g, 0.5, s_i, ALU.subtract, ALU.mult
                    )
                    t2 = tpool.tile([CH, BJ], F32, tag="t2")
                    nc.vector.tensor_tensor(t2[:], s_f, c_t[:], ALU.mult)
                    nc.vector.tensor_tensor(c_t[:], t1[:], t2[:], ALU.add)

                # ow = o * Whh_rep for the NEXT step's zm; only needs sigma
                # output, so it fills this DVE block's idle tail.
                ow = tpool.tile([CH, 32], F32, tag="ow")
                nc.vector.tensor_tensor(
                    ow[:].rearrange("p (k j) -> p k j", k=4),
                    s_o.rearrange("p (one j) -> p one j", one=1).broadcast_to(
                        (CH, 4, BJ)
                    ),
                    whh_t[:].rearrange("p (k j) -> p k j", k=4),
                    ALU.mult,
                )

                tc_t = tpool.tile([CH, BJ], F32, tag="tc")
                nc.scalar.activation(tc_t[:], c_t[:], AF.Tanh, scale=2.0)

                h_pending = (h_v[:, :, tl : tl + 1], s_o, tc_t)
                tc_prev, ow_prev = tc_t, ow

            dma_pending = (out_d[:, :, t0 : t0 + tb], h_v)

        # drain the last step's h and the last block's DMA
        ph_slice, ps_o, ptc = h_pending
        nc.vector.tensor_tensor(
            ph_slice, ps_o.rearrange("p (j one) -> p j one", one=1),
            ptc[:].rearrange("p (j one) -> p j one", one=1),
            ALU.mult,
        )
        pout, ph_v = dma_pending
        nc.sync.dma_start(pout, ph_v)

    nc.compile()
    return nc


def _build_wrep(w4):
    """w4: [CH, 4] gate order [i, f, g, o] -> [CH, 32] with col k*8+j holding
    the gate-k weight (j-independent), col gate order [i, f, o, g], g doubled
    for the tanh-to-sigmoid transform."""
    cols = np.stack(
        [w4[:, 0], w4[:, 1], w4[:, 3], 2.0 * w4[:, 2]], axis=1
    )  # [CH, 4]
    return np.ascontiguousarray(np.repeat(cols, BJ, axis=1).astype(np.float32))


def kernel(x, W_ih, W_hh):
    x = np.asarray(x, np.float32)
    W_ih = np.asarray(W_ih, np.float32)
    W_hh = np.asarray(W_hh, np.float32)

    key = ("nc", T, TB)
    if key not in _CACHE:
        _CACHE[key] = build_nc(T, TB)
    nc = _CACHE[key]

    in_maps = []
    for core in range(N_CORES):
        cb, bb = divmod(core, 4)
        c0, b0 = cb * CH, bb * BJ
        xt = np.ascontiguousarray(
            x[b0 : b0 + BJ, c0 : c0 + CH, :].transpose(1, 0, 2)
        )
        in_maps.append(
            {
                "xt": xt,
                "wih": _build_wrep(W_ih[c0 : c0 + CH]),
                "whh": _build_wrep(W_hh[c0 : c0 + CH]),
            }
        )

    res = run_bass_kernel_spmd(nc, in_maps, list(range(N_CORES)))

    out = np.empty((B, C, T), np.float32)
    for core in range(N_CORES):
        cb, bb = divmod(core, 4)
        c0, b0 = cb * CH, bb * BJ
        out[b0 : b0 + BJ, c0 : c0 + CH, :] = res.results[core]["out"].transpose(
            1, 0, 2
        )
    return out


# revision 17
# speedup vs baseline: 2307.4736x; 1.0345x over previous
"""DepthLSTM Trainium2 kernel (scheme F: gates along the free dimension).

Problem: x (32, 256, 4096) f32; per-channel scalar LSTM (input_size=1,
hidden_size=1, no bias), gate order [i, f, g, o], weights W_ih/W_hh (256, 4).
Output h for every timestep: (32, 256, 4096).

Sharding: 8 cores as (channel-block, batch-block) = (2 x 4): core idx
(cb, bb) owns channels [128*cb, 128*cb+128) and batches [8*bb, 8*bb+8).

Per-core layout: partitions = 128 channels; free dim packs (gate k, batch j)
as col k*8+j, gate order [i, f, o, g]. No TensorE/PSUM at all -- the gate
pre-activation z_t = A_t + W_hh * h_{t-1} is two VectorE ops (a broadcast
tensor_tensor against a host-replicated [128, 32] weight tile, then an add
against the bulk-precomputed A = x * W_ih).

Per step (all tiles SBUF, all ops full 128 partitions):
  sig:  s = sigmoid(z)            one ScalarE op over all 4 gate groups;
                                  the g cols hold sigma(2*zg) because the
                                  host doubles the g-gate weights
                                  (tanh(x) = 2*sigma(2x) - 1).
  cell: t1 = (sg - 0.5) * i       scalar_tensor_tensor; equals i*g/2
        t2 = f * c'               with rescaled state c' = c/2
        c' = t1 + t2
  out:  tc = tanh(2*c') = tanh(c) ScalarE with scale=2
        h  = o * tc               written into the h history tile (also the
                                  DMA staging buffer and the z rhs)
  z':   zm = h_bcast * Whh_rep    stride-0 broadcast of h over the 4 gates
        z  = zm + A_{t+1}

A = x * W_ih is precomputed per T-block, split across VectorE (i, f gates)
and ScalarE (o, g gates) in quarter-block chunks so the serial per-step
dependency chain is never blocked behind a long bulk op.
"""

import sys

sys.path.insert(0, "/opt/trn_rl_repo")

from contextlib import ExitStack

import numpy as np

import concourse.bacc as bacc
import concourse.tile as tile
from concourse import mybir
from concourse.bass_utils import run_bass_kernel_spmd

F32 = mybir.dt.float32
AF = mybir.ActivationFunctionType
ALU = mybir.AluOpType

B, C, T = 32, 256, 4096
N_CORES = 8
CH = 128  # channels per core
BJ = 8  # batches per core
TB = 64  # timesteps per block

_CACHE = {}


def build_nc(t_total=T, tb=TB):
    nc = bacc.Bacc("TRN2", target_bir_lowering=False, debug=False)

    x_d = nc.dram_tensor("xt", [CH, BJ, t_total], F32, kind="ExternalInput").ap()
    wih_d = nc.dram_tensor("wih", [CH, 32], F32, kind="ExternalInput").ap()
    whh_d = nc.dram_tensor("whh", [CH, 32], F32, kind="ExternalInput").ap()
    out_d = nc.dram_tensor("out", [CH, BJ, t_total], F32, kind="ExternalOutput").ap()

    n_blocks = t_total // tb

    with tile.TileContext(nc) as tc, ExitStack() as ctx:
        consts = ctx.enter_context(tc.tile_pool(name="consts", bufs=1))
        state = ctx.enter_context(tc.tile_pool(name="state", bufs=1))
        xpool = ctx.enter_context(tc.tile_pool(name="xpool", bufs=3))
        apool = ctx.enter_context(tc.tile_pool(name="apool", bufs=3))
        hpool = ctx.enter_context(tc.tile_pool(name="hpool", bufs=4))
        spool = ctx.enter_context(tc.tile_pool(name="spool", bufs=6))
        tpool = ctx.enter_context(tc.tile_pool(name="tpool", bufs=8))

        wih_t = consts.tile([CH, 32], F32)
        nc.sync.dma_start(wih_t[:], wih_d)
        whh_t = consts.tile([CH, 32], F32)
        nc.sync.dma_start(whh_t[:], whh_d)

        # Two independent half-batch chains (j 0:4 and 4:8). Each chain's
        # serial cycle is shorter (smaller free dims in every op) and the two
        # cycles interleave in each other's semaphore gaps.
        NCH = 2
        HJ = BJ // NCH  # 4 batches per chain
        c_t = [state.tile([CH, HJ], F32, tag=f"c{g}", name=f"c_state{g}") for g in range(NCH)]

        tc_prev = [None] * NCH  # tanh(c) tile from the previous step
        ow_prev = [None] * NCH  # o * Whh_rep tile from the previous step
        h_pending = [None] * NCH  # (h_slice, s_o, tc) for the previous step:
        # the h output op feeds only the DMA, so it is emitted after the next
        # step's z ops and runs during the sigma hop, off the critical path.
        dma_pending = None  # (out_slice, h_view) for a finished block

        for blk in range(n_blocks):
            t0 = blk * tb
            x_t = xpool.tile([CH, BJ * tb], F32, tag="xblk")
            nc.sync.dma_start(
                x_t[:].rearrange("p (j t) -> p j t", j=BJ),
                x_d[:, :, t0 : t0 + tb],
            )
            # x viewed as [p, t, j] to match A's (t, k, j) col order
            x_tj = x_t[:].rearrange("p (j t) -> p j t", j=BJ).transpose([0, 2, 1])

            a_t = apool.tile([CH, tb * 32], F32, tag="ablk")
            a_v = a_t[:].rearrange("p (t k j) -> p t k j", k=4, j=BJ)
            # A[:, t, k, j] = x[:, t, j] * wih[:, k*8]  (chunked, DVE + ACT)
            qt = tb // 4
            for k in range(4):
                eng = "v" if k < 2 else "a"
                for q in range(4):
                    src = x_tj[:, q * qt : (q + 1) * qt, :]
                    dst = a_v[:, q * qt : (q + 1) * qt, k, :]
                    w_col = wih_t[:, k * BJ : k * BJ + 1]
                    if eng == "v":
                        nc.vector.tensor_scalar(dst, src, w_col, None, ALU.mult)
                    else:
                        nc.scalar.activation(dst, src, AF.Copy, scale=w_col)

            h_hist = hpool.tile([CH, BJ * tb], F32, tag="hblk")
            h_v = h_hist[:].rearrange("p (j t) -> p j t", j=BJ)

            whh_v = whh_t[:].rearrange("p (k j) -> p k j", j=BJ)

            for tl in range(tb):
                t = t0 + tl
                for g in range(NCH):
                    j0 = g * HJ
                    a_slice = a_v[:, tl, :, j0 : j0 + HJ]  # [CH, 4, HJ]
                    whh_g = whh_v[:, :, j0 : j0 + HJ]

                    if t == 0:
                        z_ap = a_slice
                    else:
                        # z = h_{t-1} (bcast) * Whh + A_t, as
                        # (o_{t-1}*Whh) * tc_{t-1} + A_t: the ow product rides
                        # the previous step's first DVE block, so zm depends
                        # only on tanh(c).
                        zm = tpool.tile([CH, 4 * HJ], F32, tag=f"zm{g}")
                        tc_b = tc_prev[g][:].rearrange(
                            "p (one j) -> p one j", one=1
                        ).broadcast_to((CH, 4, HJ))
                        nc.vector.tensor_tensor(
                            zm[:].rearrange("p (k j) -> p k j", k=4),
                            tc_b,
                            ow_prev[g][:].rearrange("p (k j) -> p k j", k=4),
                            ALU.mult,
                        )
                        z_t = tpool.tile([CH, 4 * HJ], F32, tag=f"z{g}")
                        nc.vector.tensor_tensor(
                            z_t[:].rearrange("p (k j) -> p k j", k=4),
                            zm[:].rearrange("p (k j) -> p k j", k=4),
                            a_slice,
                            ALU.add,
                        )
                        z_ap = z_t[:].rearrange("p (k j) -> p k j", k=4)

                    if h_pending[g] is not None:
                        ph_slice, ps_o, ptc = h_pending[g]
                        nc.vector.tensor_tensor(
                            ph_slice,
                            ps_o.rearrange("p (j one) -> p j one", one=1),
                            ptc[:].rearrange("p (j one) -> p j one", one=1),
                            ALU.mult,
                        )
                        h_pending[g] = None
                        if g == NCH - 1 and dma_pending is not None:
                            pout, ph_v = dma_pending
                            nc.sync.dma_start(pout, ph_v)
                            dma_pending = None

                    s_t = spool.tile([CH, 4 * HJ], F32, tag=f"s{g}")
                    nc.scalar.activation(
                        s_t[:].rearrange("p (k j) -> p k j", k=4), z_ap, AF.Sigmoid
                    )
                    s_i = s_t[:, 0:HJ]
                    s_f = s_t[:, HJ : 2 * HJ]
                    s_o = s_t[:, 2 * HJ : 3 * HJ]
                    s_g = s_t[:, 3 * HJ : 4 * HJ]

                    if t == 0:
                        # c' = i * g / 2 = (sg - 0.5) * i
                        nc.vector.scalar_tensor_tensor(
                            c_t[g][:], s_g, 0.5, s_i, ALU.subtract, ALU.mult
                        )
                    else:
                        t1 = tpool.tile([CH, HJ], F32, tag=f"t1{g}")
                        nc.vector.scalar_tensor_tensor(
                            t1[:], s_g, 0.5, s_i, ALU.subtract, ALU.mult
                        )
                        t2 = tpool.tile([CH, HJ], F32, tag=f"t2{g}")
                        nc.vector.tensor_tensor(t2[:], s_f, c_t[g][:], ALU.mult)
                        nc.vector.tensor_tensor(c_t[g][:], t1[:], t2[:], ALU.add)

                    # ow = o * Whh for the NEXT step's zm; fills this DVE
                    # block's idle tail.
                    ow = tpool.tile([CH, 4 * HJ], F32, tag=f"ow{g}")
                    nc.vector.tensor_tensor(
                        ow[:].rearrange("p (k j) -> p k j", k=4),
                        s_o.rearrange("p (one j) -> p one j", one=1).broadcast_to(
                            (CH, 4, HJ)
                        ),
                        whh_g,
                        ALU.mult,
                    )

                    tc_t = tpool.tile([CH, HJ], F32, tag=f"tc{g}")
                    nc.scalar.activation(tc_t[:], c_t[g][:], AF.Tanh, scale=2.0)

                    h_pending[g] = (h_v[:, j0 : j0 + HJ, tl : tl + 1], s_o, tc_t)
                    tc_prev[g], ow_prev[g] = tc_t, ow

            dma_pending = (out_d[:, :, t0 : t0 + tb], h_v)

        # drain the last step's h ops and the last block's DMA
        for g in range(NCH):
            ph_slice, ps_o, ptc = h_pending[g]
            nc.vector.tensor_tensor(
                ph_slice, ps_o.rearrange("p (j one) -> p j one", one=1),
                ptc[:].rearrange("p (j one) -> p j one", one=1),
                ALU.mult,
            )
        pout, ph_v = dma_pending
        nc.sync.dma_start(pout, ph_v)

    nc.compile()
    return nc


def _build_wrep(w4):
    """w4: [CH, 4] gate order [i, f, g, o] -> [CH, 32] with col k*8+j holding
    the gate-k weight (j-independent), col gate order [i, f, o, g], g doubled
    for the tanh-to-sigmoid transform."""
    cols = np.stack(
        [w4[:, 0], w4[:, 1], w4[:, 3], 2.0 * w4[:, 2]], axis=1
    )  # [CH, 4]
    return np.ascontiguousarray(np.repeat(cols, BJ, axis=1).astype(np.float32))


def kernel(x, W_ih, W_hh):
    x = np.asarray(x, np.float32)
    W_ih = np.asarray(W_ih, np.float32)
    W_hh = np.asarray(W_hh, np.float32)

    key = ("nc", T, TB)
    if key not in _CACHE:
        _CACHE[key] = build_nc(T, TB)
    nc = _CACHE[key]

    in_maps = []
    for core in range(N_CORES):
        cb, bb = divmod(core, 4)
        c0, b0 = cb * CH, bb * BJ
        xt = np.ascontiguousarray(
            x[b0 : b0 + BJ, c0 : c0 + CH, :].transpose(1, 0, 2)
        )
        in_maps.append(
            {
                "xt": xt,
                "wih": _build_wrep(W_ih[c0 : c0 + CH]),
                "whh": _build_wrep(W_hh[c0 : c0 + CH]),
            }
        )

    res = run_bass_kernel_spmd(nc, in_maps, list(range(N_CORES)))

    out = np.empty((B, C, T), np.float32)
    for core in range(N_CORES):
        cb, bb = divmod(core, 4)
        c0, b0 = cb * CH, bb * BJ
        out[b0 : b0 + BJ, c0 : c0 + CH, :] = res.results[core]["out"].transpose(
            1, 0, 2
        )
    return out


# revision 18
# speedup vs baseline: 2309.8744x; 1.0010x over previous
"""DepthLSTM Trainium2 kernel (scheme F: gates along the free dimension).

Problem: x (32, 256, 4096) f32; per-channel scalar LSTM (input_size=1,
hidden_size=1, no bias), gate order [i, f, g, o], weights W_ih/W_hh (256, 4).
Output h for every timestep: (32, 256, 4096).

Sharding: 8 cores as (channel-block, batch-block) = (2 x 4): core idx
(cb, bb) owns channels [128*cb, 128*cb+128) and batches [8*bb, 8*bb+8).

Per-core layout: partitions = 128 channels; free dim packs (gate k, batch j)
as col k*8+j, gate order [i, f, o, g]. No TensorE/PSUM at all -- the gate
pre-activation z_t = A_t + W_hh * h_{t-1} is two VectorE ops (a broadcast
tensor_tensor against a host-replicated [128, 32] weight tile, then an add
against the bulk-precomputed A = x * W_ih).

Per step (all tiles SBUF, all ops full 128 partitions):
  sig:  s = sigmoid(z)            one ScalarE op over all 4 gate groups;
                                  the g cols hold sigma(2*zg) because the
                                  host doubles the g-gate weights
                                  (tanh(x) = 2*sigma(2x) - 1).
  cell: t1 = (sg - 0.5) * i       scalar_tensor_tensor; equals i*g/2
        t2 = f * c'               with rescaled state c' = c/2
        c' = t1 + t2
  out:  tc = tanh(2*c') = tanh(c) ScalarE with scale=2
        h  = o * tc               written into the h history tile (also the
                                  DMA staging buffer and the z rhs)
  z':   zm = h_bcast * Whh_rep    stride-0 broadcast of h over the 4 gates
        z  = zm + A_{t+1}

A = x * W_ih is precomputed per T-block, split across VectorE (i, f gates)
and ScalarE (o, g gates) in quarter-block chunks so the serial per-step
dependency chain is never blocked behind a long bulk op.
"""

import sys

sys.path.insert(0, "/opt/trn_rl_repo")

from contextlib import ExitStack

import numpy as np

import concourse.bacc as bacc
import concourse.tile as tile
from concourse import mybir
from concourse.bass_utils import run_bass_kernel_spmd

F32 = mybir.dt.float32
AF = mybir.ActivationFunctionType
ALU = mybir.AluOpType

B, C, T = 32, 256, 4096
N_CORES = 8
CH = 128  # channels per core
BJ = 8  # batches per core
TB = 64  # timesteps per block

_CACHE = {}


def build_nc(t_total=T, tb=TB):
    nc = bacc.Bacc("TRN2", target_bir_lowering=False, debug=False)

    x_d = nc.dram_tensor("xt", [CH, BJ, t_total], F32, kind="ExternalInput").ap()
    wih_d = nc.dram_tensor("wih", [CH, 32], F32, kind="ExternalInput").ap()
    whh_d = nc.dram_tensor("whh", [CH, 32], F32, kind="ExternalInput").ap()
    out_d = nc.dram_tensor("out", [CH, BJ, t_total], F32, kind="ExternalOutput").ap()

    n_blocks = t_total // tb

    with tile.TileContext(nc) as tc, ExitStack() as ctx:
        consts = ctx.enter_context(tc.tile_pool(name="consts", bufs=1))
        state = ctx.enter_context(tc.tile_pool(name="state", bufs=1))
        xpool = ctx.enter_context(tc.tile_pool(name="xpool", bufs=3))
        apool = ctx.enter_context(tc.tile_pool(name="apool", bufs=3))
        hpool = ctx.enter_context(tc.tile_pool(name="hpool", bufs=4))
        spool = ctx.enter_context(tc.tile_pool(name="spool", bufs=6))
        tpool = ctx.enter_context(tc.tile_pool(name="tpool", bufs=8))

        wih_t = consts.tile([CH, 32], F32)
        nc.sync.dma_start(wih_t[:], wih_d)
        whh_t = consts.tile([CH, 32], F32)
        nc.sync.dma_start(whh_t[:], whh_d)

        # Two independent half-batch chains (j 0:4 and 4:8). Each chain's
        # serial cycle is shorter (smaller free dims in every op) and the two
        # cycles interleave in each other's semaphore gaps.
        NCH = 2
        HJ = BJ // NCH  # 4 batches per chain
        c_t = [state.tile([CH, HJ], F32, tag=f"c{g}", name=f"c_state{g}") for g in range(NCH)]

        tc_prev = [None] * NCH  # tanh(c) tile from the previous step
        ow_prev = [None] * NCH  # o * Whh_rep tile from the previous step
        h_pending = [None] * NCH  # (h_slice, s_o, tc) for the previous step:
        # the h output op feeds only the DMA, so it is emitted after the next
        # step's z ops and runs during the sigma hop, off the critical path.
        dma_pending = None  # (out_slice, h_view) for a finished block

        for blk in range(n_blocks):
            t0 = blk * tb
            x_t = xpool.tile([CH, BJ * tb], F32, tag="xblk")
            nc.sync.dma_start(
                x_t[:].rearrange("p (j t) -> p j t", j=BJ),
                x_d[:, :, t0 : t0 + tb],
            )
            # x viewed as [p, t, j] to match A's (t, k, j) col order
            x_tj = x_t[:].rearrange("p (j t) -> p j t", j=BJ).transpose([0, 2, 1])

            a_t = apool.tile([CH, tb * 32], F32, tag="ablk")
            a_v = a_t[:].rearrange("p (t k j) -> p t k j", k=4, j=BJ)
            # A[:, t, k, j] = x[:, t, j] * wih[:, k*8]  (chunked, DVE + ACT)
            qt = tb // 2
            for k in range(4):
                eng = "v" if k < 2 else "a"
                for q in range(2):
                    src = x_tj[:, q * qt : (q + 1) * qt, :]
                    dst = a_v[:, q * qt : (q + 1) * qt, k, :]
                    w_col = wih_t[:, k * BJ : k * BJ + 1]
                    if eng == "v":
                        nc.vector.tensor_scalar(dst, src, w_col, None, ALU.mult)
                    else:
                        nc.scalar.activation(dst, src, AF.Copy, scale=w_col)

            h_hist = hpool.tile([CH, BJ * tb], F32, tag="hblk")
            h_v = h_hist[:].rearrange("p (j t) -> p j t", j=BJ)

            whh_v = whh_t[:].rearrange("p (k j) -> p k j", j=BJ)

            for tl in range(tb):
                t = t0 + tl
                for g in range(NCH):
                    j0 = g * HJ
                    a_slice = a_v[:, tl, :, j0 : j0 + HJ]  # [CH, 4, HJ]
                    whh_g = whh_v[:, :, j0 : j0 + HJ]

                    if t == 0:
                        z_ap = a_slice
                    else:
                        # z = h_{t-1} (bcast) * Whh + A_t, as
                        # (o_{t-1}*Whh) * tc_{t-1} + A_t: the ow product rides
                        # the previous step's first DVE block, so zm depends
                        # only on tanh(c).
                        zm = tpool.tile([CH, 4 * HJ], F32, tag=f"zm{g}")
                        tc_b = tc_prev[g][:].rearrange(
                            "p (one j) -> p one j", one=1
                        ).broadcast_to((CH, 4, HJ))
                        nc.vector.tensor_tensor(
                            zm[:].rearrange("p (k j) -> p k j", k=4),
                            tc_b,
                            ow_prev[g][:].rearrange("p (k j) -> p k j", k=4),
                            ALU.mult,
                        )
                        z_t = tpool.tile([CH, 4 * HJ], F32, tag=f"z{g}")
                        nc.vector.tensor_tensor(
                            z_t[:].rearrange("p (k j) -> p k j", k=4),
                            zm[:].rearrange("p (k j) -> p k j", k=4),
                            a_slice,
                            ALU.add,
                        )
                        z_ap = z_t[:].rearrange("p (k j) -> p k j", k=4)

                    if h_pending[g] is not None:
                        ph_slice, ps_o, ptc = h_pending[g]
                        nc.vector.tensor_tensor(
                            ph_slice,
                            ps_o.rearrange("p (j one) -> p j one", one=1),
                            ptc[:].rearrange("p (j one) -> p j one", one=1),
                            ALU.mult,
                        )
                        h_pending[g] = None
                        if g == NCH - 1 and dma_pending is not None:
                            pout, ph_v = dma_pending
                            nc.sync.dma_start(pout, ph_v)
                            dma_pending = None

                    s_t = spool.tile([CH, 4 * HJ], F32, tag=f"s{g}")
                    nc.scalar.activation(
                        s_t[:].rearrange("p (k j) -> p k j", k=4), z_ap, AF.Sigmoid
                    )
                    s_i = s_t[:, 0:HJ]
                    s_f = s_t[:, HJ : 2 * HJ]
                    s_o = s_t[:, 2 * HJ : 3 * HJ]
                    s_g = s_t[:, 3 * HJ : 4 * HJ]

                    if t == 0:
                        # c' = i * g / 2 = (sg - 0.5) * i
                        nc.vector.scalar_tensor_tensor(
                            c_t[g][:], s_g, 0.5, s_i, ALU.subtract, ALU.mult
                        )
                    else:
                        t1 = tpool.tile([CH, HJ], F32, tag=f"t1{g}")
                        nc.vector.scalar_tensor_tensor(
                            t1[:], s_g, 0.5, s_i, ALU.subtract, ALU.mult
                        )
                        t2 = tpool.tile([CH, HJ], F32, tag=f"t2{g}")
                        nc.vector.tensor_tensor(t2[:], s_f, c_t[g][:], ALU.mult)
                        nc.vector.tensor_tensor(c_t[g][:], t1[:], t2[:], ALU.add)

                    # ow = o * Whh for the NEXT step's zm; fills this DVE
                    # block's idle tail.
                    ow = tpool.tile([CH, 4 * HJ], F32, tag=f"ow{g}")
                    nc.vector.tensor_tensor(
                        ow[:].rearrange("p (k j) -> p k j", k=4),
                        s_o.rearrange("p (one j) -> p one j", one=1).broadcast_to(
                            (CH, 4, HJ)
                        ),
                        whh_g,
                        ALU.mult,
                    )

                    tc_t = tpool.tile([CH, HJ], F32, tag=f"tc{g}")
                    nc.scalar.activation(tc_t[:], c_t[g][:], AF.Tanh, scale=2.0)

                    h_pending[g] = (h_v[:, j0 : j0 + HJ, tl : tl + 1], s_o, tc_t)
                    tc_prev[g], ow_prev[g] = tc_t, ow

            dma_pending = (out_d[:, :, t0 : t0 + tb], h_v)

        # drain the last step's h ops and the last block's DMA
        for g in range(NCH):
            ph_slice, ps_o, ptc = h_pending[g]
            nc.vector.tensor_tensor(
                ph_slice, ps_o.rearrange("p (j one) -> p j one", one=1),
                ptc[:].rearrange("p (j one) -> p j one", one=1),
                ALU.mult,
            )
        pout, ph_v = dma_pending
        nc.sync.dma_start(pout, ph_v)

    nc.compile()
    return nc


def _build_wrep(w4):
    """w4: [CH, 4] gate order [i, f, g, o] -> [CH, 32] with col k*8+j holding
    the gate-k weight (j-independent), col gate order [i, f, o, g], g doubled
    for the tanh-to-sigmoid transform."""
    cols = np.stack(
        [w4[:, 0], w4[:, 1], w4[:, 3], 2.0 * w4[:, 2]], axis=1
    )  # [CH, 4]
    return np.ascontiguousarray(np.repeat(cols, BJ, axis=1).astype(np.float32))


def kernel(x, W_ih, W_hh):
    x = np.asarray(x, np.float32)
    W_ih = np.asarray(W_ih, np.float32)
    W_hh = np.asarray(W_hh, np.float32)

    key = ("nc", T, TB)
    if key not in _CACHE:
        _CACHE[key] = build_nc(T, TB)
    nc = _CACHE[key]

    in_maps = []
    for core in range(N_CORES):
        cb, bb = divmod(core, 4)
        c0, b0 = cb * CH, bb * BJ
        xt = np.ascontiguousarray(
            x[b0 : b0 + BJ, c0 : c0 + CH, :].transpose(1, 0, 2)
        )
        in_maps.append(
            {
                "xt": xt,
                "wih": _build_wrep(W_ih[c0 : c0 + CH]),
                "whh": _build_wrep(W_hh[c0 : c0 + CH]),
            }
        )

    res = run_bass_kernel_spmd(nc, in_maps, list(range(N_CORES)))

    out = np.empty((B, C, T), np.float32)
    for core in range(N_CORES):
        cb, bb = divmod(core, 4)
        c0, b0 = cb * CH, bb * BJ
        out[b0 : b0 + BJ, c0 : c0 + CH, :] = res.results[core]["out"].transpose(
            1, 0, 2
        )
    return out
